# revision 1
# baseline (speedup 1.0000x reference)
"""Multi-head attention (S=4096, D=1024, H=16, dk=dv=64) on 8 trn2 NeuronCores.

Sharding: tensor-parallel over heads — 2 heads per core. Each core receives
the (host-transposed, bf16-cast) activations plus its two heads' projection
weights and its 128-column slice of Wo, computes its heads' attention and a
partial output product y_c = hc @ Wo[:, c-slice].T, and the host all-reduces
the 8 partials (the "row-shard W_o and all-reduce" variant, with the
all-reduce in the gather step).

Per-core Bass/Tile kernel (engines execute their streams in order, so the
program is software-pipelined by emission order):
  - Projections qhT/khT [128,S] (head A on partitions 0-63, head B on 64-127)
    and vh tiles [sk, dv]+ones-column (PE-transposed), streamed per 512-wide
    strip and interleaved into strip 0's attention loop (DMA-paced).
  - Attention, ACT-paced steady state: per sk tile, scores^T for both heads
    concurrently (PE row groups at base partitions 0/64), one Exp over the
    [128,1024] PSUM pair (scale=1/8; no max subtraction — scores are O(1)),
    two PV accumulations out^T[dv+1, sq]; the ones column yields the softmax
    denominator in row 64.
  - Per-strip epilogue, drip-fed into the NEXT strip's loop: one reciprocal
    over both denominator rows (adjacent partitions via an SBUF->SBUF DMA
    partition move), K=2 selector-matmul broadcast across 64 partitions
    (DVE cannot cross partitions), in-place normalize, head B shifted under
    head A by DMA, then single K=128 output-projection matmuls.

Matmul operands are bf16 (full PE rate; fp32 is 1/4 rate and fp32r is an
e11m8 format with the same 8-bit mantissa). PSUM accumulation is fp32; exp
input is exact fp32; denominators stay fp32 through the reciprocal.
PSUM (8 banks): qk pairs 2x[128,1024]=4, pv 2x[128,512]=2, post 2x[128,512]=2.
"""

import contextlib
import sys

if "/opt/trn_rl_repo" not in sys.path:
    sys.path.insert(0, "/opt/trn_rl_repo")

import numpy as np
import ml_dtypes

import concourse.bass as bass
import concourse.mybir as mybir
import concourse.tile as tile
from concourse.masks import make_identity

F32 = mybir.dt.float32
BF16 = mybir.dt.bfloat16
EXP = mybir.ActivationFunctionType.Exp
MULT = mybir.AluOpType.mult

S, D, DK, P, W = 4096, 1024, 64, 128, 512
NS = S // W      # 8 sq strips
NT = S // P      # 32 sk tiles
NDC = D // P     # 8 contraction chunks for projections
TPW = W // P     # sk tiles per strip (4)
SCALE = 0.125    # 1/sqrt(DK)
NCORES = 8


def _split_excess_waits(nc, max_waits=1, max_waits_evsem=2):
    """The walrus build in this container rejects instructions carrying more
    than ~2 sync-wait commands; Tile's exit drain aggregates one wait per live
    semaphore onto single instructions. Split the excess onto preceding NoOps
    on the same engine (engine streams are in-order, so semantics hold)."""
    for fn in nc.m.functions:
        for blk in fn.blocks:
            new_insts = []
            for inst in blk.instructions:
                si = getattr(inst, "sync_info", None)
                lim = (
                    max_waits_evsem
                    if isinstance(inst, mybir.InstEventSemaphore)
                    else max_waits
                )
                if si is not None and si.on_wait and len(si.on_wait) > lim:
                    waits = list(si.on_wait)
                    for w in waits[:-lim]:
                        new_insts.append(
                            mybir.InstNoOp(
                                name=nc.get_next_instruction_name(),
                                engine=inst.engine,
                                bass_nofuse=True,
                                sync_info=mybir.SyncInfo(on_wait=[w], on_update=[]),
                            )
                        )
                    si.on_wait = waits[-lim:]
                new_insts.append(inst)
            blk.instructions = new_insts


def _build_mha(nc: bass.Bass):
    qT = nc.dram_tensor("qT", [D, S], BF16, kind="ExternalInput")
    kT = nc.dram_tensor("kT", [D, S], BF16, kind="ExternalInput")
    vT = nc.dram_tensor("vT", [D, S], BF16, kind="ExternalInput")
    wq = nc.dram_tensor("wq", [D, P], BF16, kind="ExternalInput")
    wk = nc.dram_tensor("wk", [D, P], BF16, kind="ExternalInput")
    wv = nc.dram_tensor("wv", [D, P], BF16, kind="ExternalInput")
    wo = nc.dram_tensor("wo", [P, D], BF16, kind="ExternalInput")
    seld = nc.dram_tensor("seld", [2, 2 * DK], BF16, kind="ExternalInput")
    y = nc.dram_tensor("y", [S, D], F32, kind="ExternalOutput")

    qT3 = qT.rearrange("(o p) s -> p o s", p=P)
    kT3 = kT.rearrange("(o p) s -> p o s", p=P)
    vT3 = vT.rearrange("(o p) s -> p o s", p=P)
    wq3 = wq.rearrange("(o p) m -> p o m", p=P)
    wk3 = wk.rearrange("(o p) m -> p o m", p=P)
    wv3 = wv.rearrange("(o p) m -> p o m", p=P)

    with tile.TileContext(nc) as tc, contextlib.ExitStack() as ctx:
        static = ctx.enter_context(tc.tile_pool(name="static", bufs=1))
        xpool = ctx.enter_context(tc.tile_pool(name="x", bufs=8))
        vtmp = ctx.enter_context(tc.tile_pool(name="vtmp", bufs=2))
        ptp = ctx.enter_context(tc.tile_pool(name="pt", bufs=6))
        recp = ctx.enter_context(tc.tile_pool(name="rec", bufs=2))
        ystage = ctx.enter_context(tc.tile_pool(name="ystage", bufs=4))
        qk_ps = ctx.enter_context(tc.tile_pool(name="qkps", bufs=2, space="PSUM"))
        pv_ps = ctx.enter_context(tc.tile_pool(name="pvps", bufs=2, space="PSUM"))
        post_ps = ctx.enter_context(tc.tile_pool(name="postps", bufs=2, space="PSUM"))

        wq_sb = static.tile([P, NDC, P], BF16, tag="wq")
        wk_sb = static.tile([P, NDC, P], BF16, tag="wk")
        wv_sb = static.tile([P, NDC, P], BF16, tag="wv")
        wo_sb = static.tile([P, D], BF16, tag="wo")
        ident = static.tile([P, P], BF16, tag="ident")
        sel = static.tile([DK + 2, 2 * DK], BF16, tag="sel")
        khT = static.tile([P, S], BF16, tag="khT")
        qhT = static.tile([P, S], BF16, tag="qhT")
        vh = static.tile([P, NT, 2 * DK + 2], BF16, tag="vh")
        hc = static.tile([P, S], BF16, tag="hc")
        tmpb_pool = recp

        nc.sync.dma_start(wq_sb[:], wq3)
        nc.sync.dma_start(wk_sb[:], wk3)
        nc.sync.dma_start(wv_sb[:], wv3)
        make_identity(nc, ident[:])
        # HAM warm-up: ~4us of dummy PE work (no DMA dependency) so the
        # projections hit the array already at 2.4 GHz.
        warm = post_ps.tile([P, W], F32, tag="post")
        for _ in range(40):
            nc.tensor.matmul(warm[:, 0:P], ident[:], ident[:], start=True, stop=True)

        def one_proj(jw, w_sb, src3, dst):
            def _th():
                xx = xpool.tile([P, NDC, W], BF16, tag="xs")
                nc.sync.dma_start(xx[:], src3[:, :, jw])
                pp = post_ps.tile([P, W], F32, tag="post")
                for c in range(NDC):
                    nc.tensor.matmul(
                        pp[:], w_sb[:, c, :], xx[:, c, :],
                        start=(c == 0), stop=(c == NDC - 1),
                    )
                nc.vector.tensor_copy(dst, pp[:])
            return _th

        def proj_thunks(j):
            """Projection strip j as 4 thunks: q, k, v, v-transposes."""
            jw = slice(j * W, (j + 1) * W)
            vts = vtmp.tile([P, W], BF16, tag="vts")

            def _trs():
                for i in range(TPW):
                    t = j * TPW + i
                    ptr = post_ps.tile([P, P], BF16, tag="post")
                    nc.tensor.transpose(ptr[:], vts[:, i * P : (i + 1) * P], ident[:])
                    # ptr rows = sk; cols 0:64 head A dv, 64:128 head B dv
                    nc.vector.tensor_copy(vh[:, t, 0:DK], ptr[:, 0:DK])
                    nc.vector.tensor_copy(
                        vh[:, t, DK + 1 : 2 * DK + 1], ptr[:, DK : 2 * DK]
                    )
                    nc.gpsimd.memset(vh[:, t, DK : DK + 1], 1.0)
                    nc.gpsimd.memset(vh[:, t, 2 * DK + 1 : 2 * DK + 2], 1.0)

            return [
                one_proj(jw, wq_sb, qT3, qhT[:, jw]),
                one_proj(jw, wk_sb, kT3, khT[:, jw]),
                one_proj(jw, wv_sb, vT3, vts[:]),
                _trs,
            ]

        def emit_proj(j):
            for th in proj_thunks(j):
                th()

        def make_epilogue(s, rec, tmpb):
            """Deferred post-softmax work for strip s, drip-fed into the next
            strip's attention loop (fills PE slack under the exp pace)."""
            cw = slice(s * W, (s + 1) * W)
            thunks = []

            def norm_a():
                bc = post_ps.tile([P, W], F32, tag="post")
                nc.tensor.matmul(
                    bc[0:DK, :], sel[DK : DK + 2, 0:DK], rec[DK : DK + 2, :],
                    start=True, stop=True,
                )
                nc.vector.tensor_tensor(
                    hc[0:DK, cw], hc[0:DK, cw], bc[0:DK, :], op=MULT
                )

            def norm_b():
                bc = post_ps.tile([P, W], F32, tag="post")
                nc.tensor.matmul(
                    bc[0:DK, :], sel[DK : DK + 2, DK : 2 * DK], rec[DK : DK + 2, :],
                    start=True, stop=True,
                )
                nc.vector.tensor_tensor(
                    tmpb[0:DK, :], tmpb[0:DK, :], bc[0:DK, :], op=MULT
                )
                nc.sync.dma_start(hc[DK:P, cw], tmpb[0:DK, :])

            thunks.append(norm_a)
            thunks.append(norm_b)

            def proj_out(i, oh):
                def _th():
                    sq = s * TPW + i
                    py = post_ps.tile([P, W], F32, tag="post")
                    nc.tensor.matmul(
                        py[:],
                        hc[:, sq * P : (sq + 1) * P],
                        wo_sb[:, oh * W : (oh + 1) * W],
                        start=True, stop=True,
                    )
                    ys = ystage.tile([P, W], F32, tag="ys")
                    # final strip: exp stream is over, ScalarE is idle — let it
                    # share the PSUM evacuations so the tail chain is shorter
                    if s == NS - 1 and oh == 1:
                        nc.scalar.copy(ys[:], py[:])
                    else:
                        nc.vector.tensor_copy(ys[:], py[:])
                    nc.sync.dma_start(
                        y[sq * P : (sq + 1) * P, oh * W : (oh + 1) * W], ys[:]
                    )
                return _th

            for i in range(TPW):
                for oh in range(2):
                    thunks.append(proj_out(i, oh))
            return thunks

        # ---- main software-pipelined loop ----
        emit_proj(0)
        nc.sync.dma_start(wo_sb[:], wo[:])
        nc.sync.dma_start(sel[DK : DK + 2, :], seld[:])
        # per-strip projection thunks in dependency order; q strips deferred
        # to the end (their consumers start one strip later)
        projq = []
        qdefer = []
        for j in range(1, NS):
            q_th, k_th, v_th, trs_th = proj_thunks(j)
            projq += [k_th, v_th, trs_th]
            qdefer.append(q_th)
        projq += qdefer

        pending = []
        for s in range(NS):
            cw = slice(s * W, (s + 1) * W)
            pva = pv_ps.tile([P, W], F32, tag="pv")
            pvb = pv_ps.tile([P, W], F32, tag="pv")
            for t in range(NT):
                if s == 0 and projq:
                    projq.pop(0)()
                qk = qk_ps.tile([P, 2 * W], F32, tag="qk")
                nc.tensor.matmul(
                    qk[0:P, 0:W],
                    khT[0:DK, t * P : (t + 1) * P], qhT[0:DK, cw],
                    start=True, stop=True,
                )
                nc.tensor.matmul(
                    qk[0:P, W : 2 * W],
                    khT[DK:P, t * P : (t + 1) * P], qhT[DK:P, cw],
                    start=True, stop=True,
                )
                pt = ptp.tile([P, 2 * W], BF16, tag="pt")
                nc.scalar.activation(pt[:], qk[:], EXP, scale=SCALE)
                nc.tensor.matmul(
                    pva[0 : DK + 1, :],
                    vh[:, t, 0 : DK + 1], pt[:, 0:W],
                    start=(t == 0), stop=(t == NT - 1),
                )
                nc.tensor.matmul(
                    pvb[0 : DK + 1, :],
                    vh[:, t, DK + 1 : 2 * DK + 2], pt[:, W : 2 * W],
                    start=(t == 0), stop=(t == NT - 1),
                )
                if pending and t >= 16 and t % 2 == 0:
                    pending.pop(0)()
                    if t == NT - 2:
                        while pending:
                            pending.pop(0)()

            # strip boundary: evacuate PSUM fast (head A + head B + the two
            # denominator rows, B's moved to the adjacent partition by DMA),
            # then one reciprocal for both heads — all off the exp path.
            rs = recp.tile([P, W], F32, tag="rs")
            tmpb = tmpb_pool.tile([DK, W], BF16, tag="tmpb")
            nc.vector.tensor_copy(rs[DK : DK + 1, :], pva[DK : DK + 1, :])
            rbt = recp.tile([P, W], F32, tag="rbt")
            nc.vector.tensor_copy(rbt[DK : DK + 1, :], pvb[DK : DK + 1, :])
            nc.sync.dma_start(rs[DK + 1 : DK + 2, :], rbt[DK : DK + 1, :])
            rec = recp.tile([P, W], BF16, tag="rec")
            with nc.allow_low_precision(
                reason="bf16 softmax denominators feed a bf16 matmul broadcast"
            ):
                nc.vector.reciprocal(rec[DK : DK + 2, :], rs[DK : DK + 2, :])
            nc.vector.tensor_copy(hc[0:DK, cw], pva[0:DK, :])
            nc.vector.tensor_copy(tmpb[0:DK, :], pvb[0:DK, :])

            assert not pending
            pending = make_epilogue(s, rec, tmpb)

        for th in pending:
            th()
    return nc


def _make_core_inputs(q, k, v, Wq, Wk, Wv, Wo, core, cache):
    bf = ml_dtypes.bfloat16
    if "qT" not in cache:
        cache["qT"] = np.ascontiguousarray(q.T).astype(bf)
        cache["kT"] = np.ascontiguousarray(k.T).astype(bf)
        cache["vT"] = np.ascontiguousarray(v.T).astype(bf)
    h0, h1 = 2 * core, 2 * core + 1
    return {
        "qT": cache["qT"],
        "kT": cache["kT"],
        "vT": cache["vT"],
        "wq": np.concatenate([Wq[h0], Wq[h1]], axis=1).astype(bf),
        "wk": np.concatenate([Wk[h0], Wk[h1]], axis=1).astype(bf),
        "wv": np.concatenate([Wv[h0], Wv[h1]], axis=1).astype(bf),
        "wo": np.ascontiguousarray(Wo[:, P * core : P * (core + 1)].T).astype(bf),
        "seld": np.kron(
            np.eye(2, dtype=np.float32), np.ones((1, DK), np.float32)
        ).astype(bf),
    }


_NC = None
last_results = None  # BassKernelResults of the most recent run (for profiling)


def _get_nc():
    global _NC
    if _NC is None:
        nc = bass.Bass("TRN2", target_bir_lowering=False, debug=False)
        _build_mha(nc)
        _split_excess_waits(nc)
        _NC = nc
    return _NC


def kernel(q, k, v, Wq, Wk, Wv, Wo, **run_kwargs):
    """Full-input MHA forward. Shards over 8 NeuronCores (2 heads each),
    runs the Bass kernel, and all-reduces the per-core partial outputs."""
    from concourse.bass_utils import run_bass_kernel_spmd

    global last_results
    q = np.asarray(q, np.float32)
    k = np.asarray(k, np.float32)
    v = np.asarray(v, np.float32)
    Wq = np.asarray(Wq, np.float32)
    Wk = np.asarray(Wk, np.float32)
    Wv = np.asarray(Wv, np.float32)
    Wo = np.asarray(Wo, np.float32)

    nc = _get_nc()
    cache = {}
    in_maps = [
        _make_core_inputs(q, k, v, Wq, Wk, Wv, Wo, c, cache) for c in range(NCORES)
    ]
    res = run_bass_kernel_spmd(
        nc, in_maps, core_ids=list(range(NCORES)), **run_kwargs
    )
    last_results = res
    y = res.results[0]["y"].astype(np.float32)
    for c in range(1, NCORES):
        y += res.results[c]["y"]
    return y



# revision 6
# speedup vs baseline: 1.0218x; 1.0218x over previous
"""Multi-head attention (S=4096, D=1024, H=16, dk=dv=64) on 8 trn2 NeuronCores.

Sharding: tensor-parallel over heads — 2 heads per core. Each core receives
the (host-transposed, bf16-cast) activations plus its two heads' projection
weights and its 128-column slice of Wo, computes its heads' attention and a
partial output product y_c = hc @ Wo[:, c-slice].T, and the host all-reduces
the 8 partials (the "row-shard W_o and all-reduce" variant, with the
all-reduce in the gather step).

Per-core Bass/Tile kernel (engines execute their streams in order, so the
program is software-pipelined by emission order):
  - Projections qhT/khT [128,S] (head A on partitions 0-63, head B on 64-127)
    and vh tiles [sk, dv]+ones-column (PE-transposed), streamed per 512-wide
    strip; k/v interleaved into strip 0's attention loop (DMA-paced), q for
    strip s+1 dripped into strip s.
  - Attention: per sk tile, scores^T for both heads concurrently (PE row
    groups at base partitions 0/64) into one [128,1024] PSUM pair; softmax
    numerator split across TWO engines: most tiles take an exact Exp on
    ScalarE (scale=1/8; no max subtraction — scores are O(1)), ~10/32 tiles
    per steady strip take a one-op Schraudolph exp on VectorE
    (bits = round(scores*128*log2e/8 + 16250) as int16, bitcast to bf16 —
    2^y via the bf16 exponent field, ~±3% on those tiles only); two PV
    accumulations out^T[dv+1, sq] per tile; the ones column in vh yields the
    softmax denominator in PSUM row 64.
  - Per-strip epilogue, drip-fed into the NEXT strip's loop: denominators
    1/d = Exp(-Ln d) on ScalarE directly from the two PSUM rows (no DVE
    reciprocal, no cross-partition move), K=1 ones-matmul broadcast across
    64 partitions for each head (head B at PE col group 64 so the broadcast
    lands on partitions 64-127), in-place normalize, head B shifted under
    head A by SBUF DMA, then single K=128 output-projection matmuls.

Matmul operands are bf16 (full PE rate). PSUM accumulation is fp32; ScalarE
exp input is exact fp32; denominators stay fp32 through Ln.
PSUM (8 banks): qk pairs 2x[128,1024]=4, pv 2x[128,512]=2, post 2x[128,512]=2.
"""

import contextlib
import sys

if "/opt/trn_rl_repo" not in sys.path:
    sys.path.insert(0, "/opt/trn_rl_repo")

import numpy as np
import ml_dtypes

import concourse.bass as bass
import concourse.mybir as mybir
import concourse.tile as tile
from concourse.masks import make_identity

F32 = mybir.dt.float32
BF16 = mybir.dt.bfloat16
I16 = mybir.dt.int16
EXP = mybir.ActivationFunctionType.Exp
LN = mybir.ActivationFunctionType.Ln
MULT = mybir.AluOpType.mult
ADD = mybir.AluOpType.add

S, D, DK, P, W = 4096, 1024, 64, 128, 512
NS = S // W      # 8 sq strips
NT = S // P      # 32 sk tiles
NDC = D // P     # 8 contraction chunks for projections
TPW = W // P     # sk tiles per strip (4)
SCALE = 0.125    # 1/sqrt(DK)
NCORES = 8

# Schraudolph exp on DVE: bf16 bits of 2^(s*SCALE*log2e) ~= s*SEXP_A + SEXP_B
SEXP_A = 128.0 * SCALE * 1.4426950408889634
SEXP_B = 16256.0 - 6.0
# sk tiles per strip computed on VectorE (steady strips only)
N_DVE = 10
_DVE_TILES = frozenset(np.linspace(1, NT - 2, N_DVE, dtype=int).tolist())


def _split_excess_waits(nc, max_waits=1, max_waits_evsem=2):
    """The walrus build in this container rejects instructions carrying more
    than ~2 sync-wait commands; Tile's exit drain aggregates one wait per live
    semaphore onto single instructions. Split the excess onto preceding NoOps
    on the same engine (engine streams are in-order, so semantics hold)."""
    for fn in nc.m.functions:
        for blk in fn.blocks:
            new_insts = []
            for inst in blk.instructions:
                si = getattr(inst, "sync_info", None)
                lim = (
                    max_waits_evsem
                    if isinstance(inst, mybir.InstEventSemaphore)
                    else max_waits
                )
                if si is not None and si.on_wait and len(si.on_wait) > lim:
                    waits = list(si.on_wait)
                    for w in waits[:-lim]:
                        new_insts.append(
                            mybir.InstNoOp(
                                name=nc.get_next_instruction_name(),
                                engine=inst.engine,
                                bass_nofuse=True,
                                sync_info=mybir.SyncInfo(on_wait=[w], on_update=[]),
                            )
                        )
                    si.on_wait = waits[-lim:]
                new_insts.append(inst)
            blk.instructions = new_insts


def _build_mha(nc: bass.Bass):
    qT = nc.dram_tensor("qT", [D, S], BF16, kind="ExternalInput")
    kT = nc.dram_tensor("kT", [D, S], BF16, kind="ExternalInput")
    vT = nc.dram_tensor("vT", [D, S], BF16, kind="ExternalInput")
    wq = nc.dram_tensor("wq", [D, P], BF16, kind="ExternalInput")
    wk = nc.dram_tensor("wk", [D, P], BF16, kind="ExternalInput")
    wv = nc.dram_tensor("wv", [D, P], BF16, kind="ExternalInput")
    wo = nc.dram_tensor("wo", [P, D], BF16, kind="ExternalInput")
    y = nc.dram_tensor("y", [S, D], F32, kind="ExternalOutput")

    qT3 = qT.rearrange("(o p) s -> p o s", p=P)
    kT3 = kT.rearrange("(o p) s -> p o s", p=P)
    vT3 = vT.rearrange("(o p) s -> p o s", p=P)
    wq3 = wq.rearrange("(o p) m -> p o m", p=P)
    wk3 = wk.rearrange("(o p) m -> p o m", p=P)
    wv3 = wv.rearrange("(o p) m -> p o m", p=P)

    with tile.TileContext(nc) as tc, contextlib.ExitStack() as ctx:
        static = ctx.enter_context(tc.tile_pool(name="static", bufs=1))
        xpool = ctx.enter_context(tc.tile_pool(name="x", bufs=8))
        vtmp = ctx.enter_context(tc.tile_pool(name="vtmp", bufs=2))
        ptp = ctx.enter_context(tc.tile_pool(name="pt", bufs=6))
        recp = ctx.enter_context(tc.tile_pool(name="rec", bufs=3))
        ystage = ctx.enter_context(tc.tile_pool(name="ystage", bufs=4))
        qk_ps = ctx.enter_context(tc.tile_pool(name="qkps", bufs=2, space="PSUM"))
        pv_ps = ctx.enter_context(tc.tile_pool(name="pvps", bufs=2, space="PSUM"))
        post_ps = ctx.enter_context(tc.tile_pool(name="postps", bufs=2, space="PSUM"))

        wq_sb = static.tile([P, NDC, P], BF16, tag="wq")
        wk_sb = static.tile([P, NDC, P], BF16, tag="wk")
        wv_sb = static.tile([P, NDC, P], BF16, tag="wv")
        wo_sb = static.tile([P, D], BF16, tag="wo")
        ident = static.tile([P, P], BF16, tag="ident")
        onesb = static.tile([DK + 1, DK], BF16, tag="onesb")
        khT = static.tile([P, S], BF16, tag="khT")
        qhT = static.tile([P, S], BF16, tag="qhT")
        vh = static.tile([P, NT, 2 * DK + 2], BF16, tag="vh")
        hc = static.tile([P, S], BF16, tag="hc")

        nc.gpsimd.memset(onesb[DK : DK + 1, :], 1.0)
        nc.sync.dma_start(wq_sb[:], wq3)
        nc.sync.dma_start(wk_sb[:], wk3)
        nc.sync.dma_start(wv_sb[:], wv3)
        make_identity(nc, ident[:])
        # No explicit HAM warm-up: the strip-0 projections start as soon as
        # their DMA lands and serve as the warm-up themselves.

        def one_proj(jw, w_sb, src3, dst, half=None, xx_cache=None):
            """Projection of one 512-wide strip; half=0/1 emits only 4 of the
            8 contraction chunks (finer drip grains for strip 0's loop)."""

            def _th():
                if xx_cache is not None and xx_cache[0] is not None:
                    xx = xx_cache[0]
                else:
                    xx = xpool.tile([P, NDC, W], BF16, tag="xs")
                    nc.sync.dma_start(xx[:], src3[:, :, jw])
                    if xx_cache is not None:
                        xx_cache[0] = xx
                if xx_cache is not None and len(xx_cache) > 1 and xx_cache[1] is not None:
                    pp = xx_cache[1]
                else:
                    pp = post_ps.tile([P, W], F32, tag="post")
                    if xx_cache is not None and len(xx_cache) > 1:
                        xx_cache[1] = pp
                cs = range(NDC) if half is None else range(4 * half, 4 * half + 4)
                for c in cs:
                    nc.tensor.matmul(
                        pp[:], w_sb[:, c, :], xx[:, c, :],
                        start=(c == 0), stop=(c == NDC - 1),
                    )
                if half is None or half == 1:
                    nc.vector.tensor_copy(dst, pp[:])
            return _th

        def proj_halves(jw, w_sb, src3, dst):
            cache = [None, None]
            return [
                one_proj(jw, w_sb, src3, dst, half=0, xx_cache=cache),
                one_proj(jw, w_sb, src3, dst, half=1, xx_cache=cache),
            ]

        def v_thunks(j):
            """V projection + PE transpose for strip j, as 1+4 thunks."""
            jw = slice(j * W, (j + 1) * W)
            vts = vtmp.tile([P, W], BF16, tag="vts")

            def _tr(i):
                def _th():
                    t = j * TPW + i
                    ptr = post_ps.tile([P, P], BF16, tag="post")
                    nc.tensor.transpose(ptr[:], vts[:, i * P : (i + 1) * P], ident[:])
                    nc.vector.tensor_copy(vh[:, t, 0:DK], ptr[:, 0:DK])
                    nc.vector.tensor_copy(
                        vh[:, t, DK + 1 : 2 * DK + 1], ptr[:, DK : 2 * DK]
                    )
                return _th

            return [one_proj(jw, wv_sb, vT3, vts[:])] + [_tr(i) for i in range(TPW)]

        for t in range(NT):
            nc.gpsimd.memset(vh[:, t, DK : DK + 1], 1.0)
            nc.gpsimd.memset(vh[:, t, 2 * DK + 1 : 2 * DK + 2], 1.0)

        def q_thunks(j):
            jw = slice(j * W, (j + 1) * W)
            return proj_halves(jw, wq_sb, qT3, qhT[:, jw])

        # ---- strip 0 projections (before the loop) ----
        jw0 = slice(0, W)
        one_proj(jw0, wq_sb, qT3, qhT[:, jw0])()
        one_proj(jw0, wk_sb, kT3, khT[:, jw0])()
        for th in v_thunks(0):
            th()
        nc.sync.dma_start(wo_sb[:], wo[:])

        # strip 0 drip: k/v for strips 1-7 (needed within strip 0's loop),
        # then q for strip 1. q for strips 2-7 drip inside strips 1-6.
        drip0 = []
        for j in range(1, NS):
            jw = slice(j * W, (j + 1) * W)
            drip0 += proj_halves(jw, wk_sb, kT3, khT[:, jw])
            drip0 += v_thunks(j)
        drip0 += q_thunks(1)

        def make_epilogue(s, lnd, final):
            """Deferred post-softmax work for strip s, drip-fed into the next
            strip's attention loop (fills engine slack under the exp pace)."""
            cw = slice(s * W, (s + 1) * W)
            thunks = []
            rec = recp.tile([DK + 1, 2 * W], BF16, tag="rec")
            bch = [None]

            def rec_th():
                # 1/d for both heads in one ScalarE pass: rec = exp(-ln d)
                with nc.allow_low_precision(
                    reason="bf16 softmax denominators feed a bf16 matmul broadcast"
                ):
                    nc.scalar.activation(
                        rec[DK : DK + 1, :], lnd[DK : DK + 1, :], EXP, scale=-1.0
                    )

            def sel_th():
                bc = post_ps.tile([P, W], F32, tag="post")
                bch[0] = bc
                nc.tensor.matmul(
                    bc[0:DK, :], onesb[DK : DK + 1, :], rec[DK : DK + 1, 0:W],
                    start=True, stop=True,
                )
                nc.tensor.matmul(
                    bc[DK:P, :], onesb[DK : DK + 1, :], rec[DK : DK + 1, W : 2 * W],
                    start=True, stop=True, tile_position=(DK, DK),
                )

            def norm_a():
                nc.vector.tensor_tensor(
                    hc[0:DK, cw], hc[0:DK, cw], bch[0][0:DK, :], op=MULT
                )

            def norm_b():
                nc.vector.tensor_tensor(
                    hc[DK:P, cw], hc[DK:P, cw], bch[0][DK:P, :], op=MULT
                )

            thunks += [rec_th, sel_th, norm_a, norm_b]

            def proj_out(i, oh):
                def _th():
                    sq = s * TPW + i
                    py = post_ps.tile([P, W], F32, tag="post")
                    nc.tensor.matmul(
                        py[:],
                        hc[:, sq * P : (sq + 1) * P],
                        wo_sb[:, oh * W : (oh + 1) * W],
                        start=True, stop=True,
                    )
                    ys = ystage.tile([P, W], F32, tag="ys")
                    # final strip: exp stream is over, ScalarE is idle — let
                    # it share the PSUM evacuations so the tail is shorter
                    if final and (i + oh) % 2 == 0:
                        nc.scalar.copy(ys[:], py[:])
                    else:
                        nc.vector.tensor_copy(ys[:], py[:])
                    nc.sync.dma_start(
                        y[sq * P : (sq + 1) * P, oh * W : (oh + 1) * W], ys[:]
                    )
                return _th

            for i in range(TPW):
                for oh in range(2):
                    thunks.append(proj_out(i, oh))
            return thunks

        # ---- main software-pipelined loop ----
        pending = []
        for s in range(NS):
            cw = slice(s * W, (s + 1) * W)
            pva = pv_ps.tile([P, W], F32, tag="pv")
            pvb = pv_ps.tile([P, W], F32, tag="pv")
            dve_tiles = _DVE_TILES if s > 0 else frozenset()
            for t in range(NT):
                qk = qk_ps.tile([P, 2 * W], F32, tag="qk")
                nc.tensor.matmul(
                    qk[0:P, 0:W],
                    khT[0:DK, t * P : (t + 1) * P], qhT[0:DK, cw],
                    start=True, stop=True,
                )
                nc.tensor.matmul(
                    qk[0:P, W : 2 * W],
                    khT[DK:P, t * P : (t + 1) * P], qhT[DK:P, cw],
                    start=True, stop=True,
                )
                pt = ptp.tile([P, 2 * W], BF16, tag="pt")
                if t in dve_tiles:
                    with nc.allow_low_precision(
                        reason="schraudolph exp2 bit-trick on a minority of sk tiles"
                    ):
                        nc.vector.tensor_scalar(
                            pt[:].bitcast(I16), qk[:], SEXP_A, SEXP_B, MULT, ADD
                        )
                else:
                    nc.scalar.activation(pt[:], qk[:], EXP, scale=SCALE)
                nc.tensor.matmul(
                    pva[0 : DK + 1, :],
                    vh[:, t, 0 : DK + 1], pt[:, 0:W],
                    start=(t == 0), stop=(t == NT - 1),
                )
                nc.tensor.matmul(
                    pvb[0 : DK + 1, :],
                    vh[:, t, DK + 1 : 2 * DK + 2], pt[:, W : 2 * W],
                    start=(t == 0), stop=(t == NT - 1),
                )
                if s == 0:
                    for _ in range(2):
                        if drip0:
                            drip0.pop(0)()
                else:
                    if pending and t % 2 == 0:
                        pending.pop(0)()
                    if t == NT - 2:
                        while pending:
                            pending.pop(0)()

            # strip boundary: evacuate PSUM fast (head A direct to hc, head B
            # via a bf16 stage + partition-shift DMA), denominators via Ln on
            # ScalarE straight from the PSUM rows — all off the exp path.
            tmpb = recp.tile([DK, W], BF16, tag="tmpb")
            nc.vector.tensor_copy(hc[0:DK, cw], pva[0:DK, :])
            nc.vector.tensor_copy(tmpb[:], pvb[0:DK, :])
            nc.sync.dma_start(hc[DK:P, cw], tmpb[:])
            lnd = recp.tile([DK + 1, 2 * W], F32, tag="lnd")
            nc.scalar.activation(lnd[DK : DK + 1, 0:W], pva[DK : DK + 1, :], LN)
            nc.scalar.activation(lnd[DK : DK + 1, W : 2 * W], pvb[DK : DK + 1, :], LN)

            assert not drip0 or s == 0
            newpend = make_epilogue(s, lnd, final=(s == NS - 1))
            if s + 1 < NS - 1:
                newpend = q_thunks(s + 2) + newpend
            assert not pending
            pending = newpend

        for th in pending:
            th()
    return nc


def _make_core_inputs(q, k, v, Wq, Wk, Wv, Wo, core, cache):
    bf = ml_dtypes.bfloat16
    if "qT" not in cache:
        cache["qT"] = np.ascontiguousarray(q.T).astype(bf)
        cache["kT"] = np.ascontiguousarray(k.T).astype(bf)
        cache["vT"] = np.ascontiguousarray(v.T).astype(bf)
    h0, h1 = 2 * core, 2 * core + 1
    return {
        "qT": cache["qT"],
        "kT": cache["kT"],
        "vT": cache["vT"],
        "wq": np.concatenate([Wq[h0], Wq[h1]], axis=1).astype(bf),
        "wk": np.concatenate([Wk[h0], Wk[h1]], axis=1).astype(bf),
        "wv": np.concatenate([Wv[h0], Wv[h1]], axis=1).astype(bf),
        "wo": np.ascontiguousarray(Wo[:, P * core : P * (core + 1)].T).astype(bf),
    }


_NC = None
last_results = None  # BassKernelResults of the most recent run (for profiling)


def _get_nc():
    global _NC
    if _NC is None:
        nc = bass.Bass("TRN2", target_bir_lowering=False, debug=False)
        _build_mha(nc)
        _split_excess_waits(nc)
        _NC = nc
    return _NC


def kernel(q, k, v, Wq, Wk, Wv, Wo, **run_kwargs):
    """Full-input MHA forward. Shards over 8 NeuronCores (2 heads each),
    runs the Bass kernel, and all-reduces the per-core partial outputs."""
    from concourse.bass_utils import run_bass_kernel_spmd

    global last_results
    q = np.asarray(q, np.float32)
    k = np.asarray(k, np.float32)
    v = np.asarray(v, np.float32)
    Wq = np.asarray(Wq, np.float32)
    Wk = np.asarray(Wk, np.float32)
    Wv = np.asarray(Wv, np.float32)
    Wo = np.asarray(Wo, np.float32)

    nc = _get_nc()
    cache = {}
    in_maps = [
        _make_core_inputs(q, k, v, Wq, Wk, Wv, Wo, c, cache) for c in range(NCORES)
    ]
    res = run_bass_kernel_spmd(
        nc, in_maps, core_ids=list(range(NCORES)), **run_kwargs
    )
    last_results = res
    y = res.results[0]["y"].astype(np.float32)
    for c in range(1, NCORES):
        y += res.results[c]["y"]
    return y


# revision 11
# speedup vs baseline: 1.1596x; 1.1348x over previous
"""Multi-head attention (S=4096, D=1024, H=16, dk=dv=64) on 8 trn2 NeuronCores.

Sharding: tensor-parallel over heads — 2 heads per core. Each core receives
the (host-transposed, bf16-cast) activations plus its two heads' projection
weights and its 128-column slice of Wo, computes its heads' attention and a
partial output product y_c = hc @ Wo[:, c-slice].T, and the host all-reduces
the 8 partials (the "row-shard W_o and all-reduce" variant, with the
all-reduce in the gather step).

Per-core Bass/Tile kernel (engines execute their streams in order, so the
program is software-pipelined by emission order):
  - Projections qhT/khT [128,S] (head A on partitions 0-63, head B on 64-127)
    and vh tiles [sk, dv]+ones-column (PE-transposed), streamed per 512-wide
    strip; k/v interleaved into strip 0's attention loop (DMA-paced), q for
    strip s+1 dripped into strip s.
  - Attention: per sk tile, scores^T for both heads concurrently (PE row
    groups at base partitions 0/64) into one [128,1024] PSUM pair; softmax
    numerator split across TWO engines: most tiles take an exact Exp on
    ScalarE (scale=1/8; no max subtraction — scores are O(1)), ~10/32 tiles
    per steady strip take a one-op Schraudolph exp on VectorE
    (bits = round(scores*128*log2e/8 + 16250) as int16, bitcast to bf16 —
    2^y via the bf16 exponent field, ~±3% on those tiles only); two PV
    accumulations out^T[dv+1, sq] per tile; the ones column in vh yields the
    softmax denominator in PSUM row 64.
  - Per-strip epilogue, drip-fed into the NEXT strip's loop: denominators
    1/d = Exp(-Ln d) on ScalarE directly from the two PSUM rows (no DVE
    reciprocal, no cross-partition move), K=1 ones-matmul broadcast across
    64 partitions for each head (head B at PE col group 64 so the broadcast
    lands on partitions 64-127), in-place normalize, head B shifted under
    head A by SBUF DMA, then single K=128 output-projection matmuls.

Matmul operands are bf16 (full PE rate). PSUM accumulation is fp32; ScalarE
exp input is exact fp32; denominators stay fp32 through Ln.
PSUM (8 banks): qk pairs 2x[128,1024]=4, pv 2x[128,512]=2, post 2x[128,512]=2.
"""

import contextlib
import sys

if "/opt/trn_rl_repo" not in sys.path:
    sys.path.insert(0, "/opt/trn_rl_repo")

import numpy as np
import ml_dtypes

import concourse.bass as bass
import concourse.mybir as mybir
import concourse.tile as tile
from concourse.masks import make_identity

F32 = mybir.dt.float32
BF16 = mybir.dt.bfloat16
I16 = mybir.dt.int16
EXP = mybir.ActivationFunctionType.Exp
LN = mybir.ActivationFunctionType.Ln
MULT = mybir.AluOpType.mult
ADD = mybir.AluOpType.add

S, D, DK, P, W = 4096, 1024, 64, 128, 512
NS = S // W      # 8 sq strips
NT = S // P      # 32 sk tiles
NDC = D // P     # 8 contraction chunks for projections
TPW = W // P     # sk tiles per strip (4)
SCALE = 0.125    # 1/sqrt(DK)
NCORES = 8

# Schraudolph exp on DVE: bf16 bits of 2^(s*SCALE*log2e) ~= s*SEXP_A + SEXP_B
SEXP_A = 128.0 * SCALE * 1.4426950408889634
SEXP_B = 16256.0 - 6.0
# sk tiles per strip computed on VectorE (steady strips only)
N_DVE = 10
_DVE_TILES = frozenset(np.linspace(1, NT - 2, N_DVE, dtype=int).tolist())


def _split_excess_waits(nc, max_waits=1, max_waits_evsem=2):
    """The walrus build in this container rejects instructions carrying more
    than ~2 sync-wait commands; Tile's exit drain aggregates one wait per live
    semaphore onto single instructions. Split the excess onto preceding NoOps
    on the same engine (engine streams are in-order, so semantics hold)."""
    for fn in nc.m.functions:
        for blk in fn.blocks:
            new_insts = []
            for inst in blk.instructions:
                si = getattr(inst, "sync_info", None)
                lim = (
                    max_waits_evsem
                    if isinstance(inst, mybir.InstEventSemaphore)
                    else max_waits
                )
                if si is not None and si.on_wait and len(si.on_wait) > lim:
                    waits = list(si.on_wait)
                    for w in waits[:-lim]:
                        new_insts.append(
                            mybir.InstNoOp(
                                name=nc.get_next_instruction_name(),
                                engine=inst.engine,
                                bass_nofuse=True,
                                sync_info=mybir.SyncInfo(on_wait=[w], on_update=[]),
                            )
                        )
                    si.on_wait = waits[-lim:]
                new_insts.append(inst)
            blk.instructions = new_insts


def _build_mha(nc: bass.Bass):
    qT = nc.dram_tensor("qT", [D, S], BF16, kind="ExternalInput")
    kT = nc.dram_tensor("kT", [D, S], BF16, kind="ExternalInput")
    vT = nc.dram_tensor("vT", [D, S], BF16, kind="ExternalInput")
    wq = nc.dram_tensor("wq", [D, P], BF16, kind="ExternalInput")
    wk = nc.dram_tensor("wk", [D, P], BF16, kind="ExternalInput")
    wv = nc.dram_tensor("wv", [D, P], BF16, kind="ExternalInput")
    wo = nc.dram_tensor("wo", [P, D], BF16, kind="ExternalInput")
    y = nc.dram_tensor("y", [S, D], F32, kind="ExternalOutput")

    qT3 = qT.rearrange("(o p) s -> p o s", p=P)
    kT3 = kT.rearrange("(o p) s -> p o s", p=P)
    vT3 = vT.rearrange("(o p) s -> p o s", p=P)
    wq3 = wq.rearrange("(o p) m -> p o m", p=P)
    wk3 = wk.rearrange("(o p) m -> p o m", p=P)
    wv3 = wv.rearrange("(o p) m -> p o m", p=P)

    with tile.TileContext(nc) as tc, contextlib.ExitStack() as ctx:
        static = ctx.enter_context(tc.tile_pool(name="static", bufs=1))
        xpool = ctx.enter_context(tc.tile_pool(name="x", bufs=8))
        vtmp = ctx.enter_context(tc.tile_pool(name="vtmp", bufs=2))
        ptp = ctx.enter_context(tc.tile_pool(name="pt", bufs=6))
        recp = ctx.enter_context(tc.tile_pool(name="rec", bufs=3))
        ystage = ctx.enter_context(tc.tile_pool(name="ystage", bufs=4))
        qk_ps = ctx.enter_context(tc.tile_pool(name="qkps", bufs=2, space="PSUM"))
        pv_ps = ctx.enter_context(tc.tile_pool(name="pvps", bufs=2, space="PSUM"))
        post_ps = ctx.enter_context(tc.tile_pool(name="postps", bufs=2, space="PSUM"))

        wq_sb = static.tile([P, NDC, P], BF16, tag="wq")
        wk_sb = static.tile([P, NDC, P], BF16, tag="wk")
        wv_sb = static.tile([P, NDC, P], BF16, tag="wv")
        wo_sb = static.tile([P, D], BF16, tag="wo")
        ident = static.tile([P, P], BF16, tag="ident")
        onesb = static.tile([DK + 1, DK], BF16, tag="onesb")
        khT = static.tile([P, S], BF16, tag="khT")
        qhT = static.tile([P, S], BF16, tag="qhT")
        vh = static.tile([P, NT, 2 * DK + 2], BF16, tag="vh")
        hc = static.tile([P, S], BF16, tag="hc")

        nc.gpsimd.memset(onesb[DK : DK + 1, :], 1.0)
        nc.sync.dma_start(wq_sb[:], wq3)
        nc.sync.dma_start(wk_sb[:], wk3)
        nc.sync.dma_start(wv_sb[:], wv3)
        make_identity(nc, ident[:])
        # No explicit HAM warm-up: the strip-0 projections start as soon as
        # their DMA lands and serve as the warm-up themselves.

        def one_proj(jw, w_sb, src3, dst, half=None, xx_cache=None):
            """Projection of one 512-wide strip; half=0/1 emits only 4 of the
            8 contraction chunks (finer drip grains for strip 0's loop)."""

            def _th():
                if xx_cache is not None and xx_cache[0] is not None:
                    xx = xx_cache[0]
                else:
                    xx = xpool.tile([P, NDC, W], BF16, tag="xs")
                    nc.sync.dma_start(xx[:], src3[:, :, jw])
                    if xx_cache is not None:
                        xx_cache[0] = xx
                if xx_cache is not None and len(xx_cache) > 1 and xx_cache[1] is not None:
                    pp = xx_cache[1]
                else:
                    pp = post_ps.tile([P, W], F32, tag="post")
                    if xx_cache is not None and len(xx_cache) > 1:
                        xx_cache[1] = pp
                cs = range(NDC) if half is None else range(4 * half, 4 * half + 4)
                for c in cs:
                    nc.tensor.matmul(
                        pp[:], w_sb[:, c, :], xx[:, c, :],
                        start=(c == 0), stop=(c == NDC - 1),
                    )
                if half is None or half == 1:
                    nc.vector.tensor_copy(dst, pp[:])
            return _th

        def proj_halves(jw, w_sb, src3, dst):
            cache = [None, None]
            return [
                one_proj(jw, w_sb, src3, dst, half=0, xx_cache=cache),
                one_proj(jw, w_sb, src3, dst, half=1, xx_cache=cache),
            ]

        def v_thunks(j):
            """V projection + PE transpose for strip j, as 1+4 thunks."""
            jw = slice(j * W, (j + 1) * W)
            vts = vtmp.tile([P, W], BF16, tag="vts")

            def _tr(i):
                def _th():
                    t = j * TPW + i
                    ptr = post_ps.tile([P, P], BF16, tag="post")
                    nc.tensor.transpose(ptr[:], vts[:, i * P : (i + 1) * P], ident[:])
                    nc.vector.tensor_copy(vh[:, t, 0:DK], ptr[:, 0:DK])
                    nc.vector.tensor_copy(
                        vh[:, t, DK + 1 : 2 * DK + 1], ptr[:, DK : 2 * DK]
                    )
                return _th

            return [one_proj(jw, wv_sb, vT3, vts[:])] + [_tr(i) for i in range(TPW)]

        for t in range(NT):
            nc.gpsimd.memset(vh[:, t, DK : DK + 1], 1.0)
            nc.gpsimd.memset(vh[:, t, 2 * DK + 1 : 2 * DK + 2], 1.0)

        def q_thunks(j):
            jw = slice(j * W, (j + 1) * W)
            return proj_halves(jw, wq_sb, qT3, qhT[:, jw])

        # ---- strip 0 projections (before the loop) ----
        jw0 = slice(0, W)
        one_proj(jw0, wq_sb, qT3, qhT[:, jw0])()
        one_proj(jw0, wk_sb, kT3, khT[:, jw0])()
        for th in v_thunks(0):
            th()

        def wo_th():
            nc.sync.dma_start(wo_sb[:], wo[:])

        # strip 0 drip: k/v for strips 1-7 (needed within strip 0's loop),
        # then wo and q for strip 1. q for strips 2-7 drip inside strips 1-6.
        drip0 = []
        for j in range(1, NS):
            jw = slice(j * W, (j + 1) * W)
            drip0 += proj_halves(jw, wk_sb, kT3, khT[:, jw])
            drip0 += v_thunks(j)
            if j == 2:
                drip0.append(wo_th)
        drip0 += q_thunks(1)

        def make_epilogue(s, lnd, final):
            """Deferred post-softmax work for strip s, drip-fed into the next
            strip's attention loop (fills engine slack under the exp pace)."""
            cw = slice(s * W, (s + 1) * W)
            thunks = []
            rec = recp.tile([DK + 1, 2 * W], BF16, tag="rec")
            bch = [None]

            def rec_th():
                # 1/d for both heads in one ScalarE pass: rec = exp(-ln d)
                with nc.allow_low_precision(
                    reason="bf16 softmax denominators feed a bf16 matmul broadcast"
                ):
                    nc.scalar.activation(
                        rec[DK : DK + 1, :], lnd[DK : DK + 1, :], EXP, scale=-1.0
                    )

            def sel_th():
                bc = post_ps.tile([P, W], F32, tag="post")
                bch[0] = bc
                nc.tensor.matmul(
                    bc[0:DK, :], onesb[DK : DK + 1, :], rec[DK : DK + 1, 0:W],
                    start=True, stop=True,
                )
                nc.tensor.matmul(
                    bc[DK:P, :], onesb[DK : DK + 1, :], rec[DK : DK + 1, W : 2 * W],
                    start=True, stop=True, tile_position=(DK, DK),
                )

            def norm_a():
                nc.vector.tensor_tensor(
                    hc[0:DK, cw], hc[0:DK, cw], bch[0][0:DK, :], op=MULT
                )

            def norm_b():
                nc.vector.tensor_tensor(
                    hc[DK:P, cw], hc[DK:P, cw], bch[0][DK:P, :], op=MULT
                )

            thunks += [rec_th, sel_th, norm_a, norm_b]

            def proj_out(i, oh):
                def _th():
                    sq = s * TPW + i
                    py = post_ps.tile([P, W], F32, tag="post")
                    nc.tensor.matmul(
                        py[:],
                        hc[:, sq * P : (sq + 1) * P],
                        wo_sb[:, oh * W : (oh + 1) * W],
                        start=True, stop=True,
                    )
                    ys = ystage.tile([P, W], F32, tag="ys")
                    # final strip: exp stream is over, ScalarE is idle — let
                    # it share the PSUM evacuations so the tail is shorter
                    if final and (i + oh) % 2 == 0:
                        nc.scalar.copy(ys[:], py[:])
                    else:
                        nc.vector.tensor_copy(ys[:], py[:])
                    nc.sync.dma_start(
                        y[sq * P : (sq + 1) * P, oh * W : (oh + 1) * W], ys[:]
                    )
                return _th

            for i in range(TPW):
                for oh in range(2):
                    thunks.append(proj_out(i, oh))
            return thunks

        # ---- main software-pipelined loop ----
        # QK pairs are emitted two tiles ahead of their exp so both PSUM
        # score buffers stay full: the ScalarE and VectorE exp streams then
        # run concurrently instead of serializing on the PE's in-order queue
        # (exp_t -> PV_t -> QK_{t+1} -> exp_{t+1} would otherwise be a
        # ~1.1us dependency ring regardless of engine assignment).
        def emit_qk(cw, t):
            qk = qk_ps.tile([P, 2 * W], F32, tag="qk")
            nc.tensor.matmul(
                qk[0:P, 0:W],
                khT[0:DK, t * P : (t + 1) * P], qhT[0:DK, cw],
                start=True, stop=True,
            )
            nc.tensor.matmul(
                qk[0:P, W : 2 * W],
                khT[DK:P, t * P : (t + 1) * P], qhT[DK:P, cw],
                start=True, stop=True,
            )
            return qk

        pending = []
        qk_ahead = []
        for s in range(NS):
            cw = slice(s * W, (s + 1) * W)
            pva = pv_ps.tile([P, W], F32, tag="pv")
            pvb = pv_ps.tile([P, W], F32, tag="pv")
            dve_tiles = _DVE_TILES if s > 0 else frozenset()
            if s == 0:
                qk_ahead.append(emit_qk(cw, 0))

            def emit_pv(t, pt):
                nc.tensor.matmul(
                    pva[0 : DK + 1, :],
                    vh[:, t, 0 : DK + 1], pt[:, 0:W],
                    start=(t == 0), stop=(t == NT - 1),
                )
                nc.tensor.matmul(
                    pvb[0 : DK + 1, :],
                    vh[:, t, DK + 1 : 2 * DK + 2], pt[:, W : 2 * W],
                    start=(t == 0), stop=(t == NT - 1),
                )

            # PV emission lags two tiles so it never sits in the PE's
            # in-order queue ahead of a QK pair while waiting on its exp:
            # accumulation order into pva/pvb is commutative.
            pv_queue = []
            for t in range(NT):
                qk = qk_ahead.pop(0)
                pt = ptp.tile([P, 2 * W], BF16, tag="pt")
                if t in dve_tiles:
                    with nc.allow_low_precision(
                        reason="schraudolph exp2 bit-trick on a minority of sk tiles"
                    ):
                        nc.vector.tensor_scalar(
                            pt[:].bitcast(I16), qk[:], SEXP_A, SEXP_B, MULT, ADD
                        )
                else:
                    nc.scalar.activation(pt[:], qk[:], EXP, scale=SCALE)
                # refill the score pipeline (crossing into the next strip)
                nt = t + 1
                if nt < NT:
                    qk_ahead.append(emit_qk(cw, nt))
                elif s + 1 < NS:
                    qk_ahead.append(
                        emit_qk(slice((s + 1) * W, (s + 2) * W), 0)
                    )
                pv_queue.append((t, pt))
                if len(pv_queue) > 2:
                    emit_pv(*pv_queue.pop(0))
                if s == 0:
                    for _ in range(2):
                        if drip0:
                            drip0.pop(0)()
                else:
                    if pending and t % 2 == 0:
                        pending.pop(0)()
                    if t == NT - 2:
                        while pending:
                            pending.pop(0)()
            while pv_queue:
                emit_pv(*pv_queue.pop(0))

            # strip boundary: evacuate PSUM fast (head A direct to hc, head B
            # via a bf16 stage + partition-shift DMA), denominators via Ln on
            # ScalarE straight from the PSUM rows — all off the exp path.
            tmpb = recp.tile([DK, W], BF16, tag="tmpb")
            nc.vector.tensor_copy(hc[0:DK, cw], pva[0:DK, :])
            nc.vector.tensor_copy(tmpb[:], pvb[0:DK, :])
            nc.sync.dma_start(hc[DK:P, cw], tmpb[:])
            lnd = recp.tile([DK + 1, 2 * W], F32, tag="lnd")
            nc.scalar.activation(lnd[DK : DK + 1, 0:W], pva[DK : DK + 1, :], LN)
            nc.scalar.activation(lnd[DK : DK + 1, W : 2 * W], pvb[DK : DK + 1, :], LN)

            assert not drip0 or s == 0
            newpend = make_epilogue(s, lnd, final=(s == NS - 1))
            if s + 1 < NS - 1:
                newpend = q_thunks(s + 2) + newpend
            assert not pending
            pending = newpend

        for th in pending:
            th()
    return nc


def _make_core_inputs(q, k, v, Wq, Wk, Wv, Wo, core, cache):
    bf = ml_dtypes.bfloat16
    if "qT" not in cache:
        cache["qT"] = np.ascontiguousarray(q.T).astype(bf)
        cache["kT"] = np.ascontiguousarray(k.T).astype(bf)
        cache["vT"] = np.ascontiguousarray(v.T).astype(bf)
    h0, h1 = 2 * core, 2 * core + 1
    return {
        "qT": cache["qT"],
        "kT": cache["kT"],
        "vT": cache["vT"],
        "wq": np.concatenate([Wq[h0], Wq[h1]], axis=1).astype(bf),
        "wk": np.concatenate([Wk[h0], Wk[h1]], axis=1).astype(bf),
        "wv": np.concatenate([Wv[h0], Wv[h1]], axis=1).astype(bf),
        "wo": np.ascontiguousarray(Wo[:, P * core : P * (core + 1)].T).astype(bf),
    }


_NC = None
last_results = None  # BassKernelResults of the most recent run (for profiling)


def _get_nc():
    global _NC
    if _NC is None:
        nc = bass.Bass("TRN2", target_bir_lowering=False, debug=False)
        _build_mha(nc)
        _split_excess_waits(nc)
        _NC = nc
    return _NC


def kernel(q, k, v, Wq, Wk, Wv, Wo, **run_kwargs):
    """Full-input MHA forward. Shards over 8 NeuronCores (2 heads each),
    runs the Bass kernel, and all-reduces the per-core partial outputs."""
    from concourse.bass_utils import run_bass_kernel_spmd

    global last_results
    q = np.asarray(q, np.float32)
    k = np.asarray(k, np.float32)
    v = np.asarray(v, np.float32)
    Wq = np.asarray(Wq, np.float32)
    Wk = np.asarray(Wk, np.float32)
    Wv = np.asarray(Wv, np.float32)
    Wo = np.asarray(Wo, np.float32)

    nc = _get_nc()
    cache = {}
    in_maps = [
        _make_core_inputs(q, k, v, Wq, Wk, Wv, Wo, c, cache) for c in range(NCORES)
    ]
    res = run_bass_kernel_spmd(
        nc, in_maps, core_ids=list(range(NCORES)), **run_kwargs
    )
    last_results = res
    y = res.results[0]["y"].astype(np.float32)
    for c in range(1, NCORES):
        y += res.results[c]["y"]
    return y


# revision 16
# speedup vs baseline: 1.1748x; 1.0132x over previous
"""Multi-head attention (S=4096, D=1024, H=16, dk=dv=64) on 8 trn2 NeuronCores.

Sharding: tensor-parallel over heads — 2 heads per core. Each core receives
the (host-transposed, bf16-cast) activations plus its two heads' projection
weights and its 128-column slice of Wo, computes its heads' attention and a
partial output product y_c = hc @ Wo[:, c-slice].T, and the host all-reduces
the 8 partials (the "row-shard W_o and all-reduce" variant, with the
all-reduce in the gather step).

Per-core Bass/Tile kernel (engines execute their streams in order, so the
program is software-pipelined by emission order):
  - Projections qhT/khT [128,S] (head A on partitions 0-63, head B on 64-127)
    and vh tiles [sk, dv]+ones-column (PE-transposed), streamed per 512-wide
    strip; k/v interleaved into strip 0's attention loop (DMA-paced), q for
    strip s+1 dripped into strip s.
  - Attention: per sk tile, scores^T for both heads concurrently (PE row
    groups at base partitions 0/64) into one [128,1024] PSUM pair; softmax
    numerator split across TWO engines: most tiles take an exact Exp on
    ScalarE (scale=1/8; no max subtraction — scores are O(1)), ~10/32 tiles
    per steady strip take a one-op Schraudolph exp on VectorE
    (bits = round(scores*128*log2e/8 + 16250) as int16, bitcast to bf16 —
    2^y via the bf16 exponent field, ~±3% on those tiles only); two PV
    accumulations out^T[dv+1, sq] per tile; the ones column in vh yields the
    softmax denominator in PSUM row 64.
  - Per-strip epilogue, drip-fed into the NEXT strip's loop: denominators
    1/d = Exp(-Ln d) on ScalarE directly from the two PSUM rows (no DVE
    reciprocal, no cross-partition move), K=1 ones-matmul broadcast across
    64 partitions for each head (head B at PE col group 64 so the broadcast
    lands on partitions 64-127), in-place normalize, head B shifted under
    head A by SBUF DMA, then single K=128 output-projection matmuls.

Matmul operands are bf16 (full PE rate). PSUM accumulation is fp32; ScalarE
exp input is exact fp32; denominators stay fp32 through Ln.
PSUM (8 banks): qk pairs 2x[128,1024]=4, pv 2x[128,512]=2, post 2x[128,512]=2.
"""

import contextlib
import sys

if "/opt/trn_rl_repo" not in sys.path:
    sys.path.insert(0, "/opt/trn_rl_repo")

import numpy as np
import ml_dtypes

import concourse.bass as bass
import concourse.mybir as mybir
import concourse.tile as tile
from concourse.masks import make_identity

F32 = mybir.dt.float32
BF16 = mybir.dt.bfloat16
I16 = mybir.dt.int16
EXP = mybir.ActivationFunctionType.Exp
LN = mybir.ActivationFunctionType.Ln
MULT = mybir.AluOpType.mult
ADD = mybir.AluOpType.add

S, D, DK, P, W = 4096, 1024, 64, 128, 512
NS = S // W      # 8 sq strips
NT = S // P      # 32 sk tiles
NDC = D // P     # 8 contraction chunks for projections
TPW = W // P     # sk tiles per strip (4)
SCALE = 0.125    # 1/sqrt(DK)
NCORES = 8

# Schraudolph exp on DVE: bf16 bits of 2^(s*SCALE*log2e) ~= s*SEXP_A + SEXP_B
SEXP_A = 128.0 * SCALE * 1.4426950408889634
SEXP_B = 16256.0 - 6.0
# sk tiles per strip computed on VectorE (steady strips only)
N_DVE = 10
_DVE_TILES = frozenset(np.linspace(1, NT - 2, N_DVE, dtype=int).tolist())


def _split_excess_waits(nc, max_waits=1, max_waits_evsem=2):
    """The walrus build in this container rejects instructions carrying more
    than ~2 sync-wait commands; Tile's exit drain aggregates one wait per live
    semaphore onto single instructions. Split the excess onto preceding NoOps
    on the same engine (engine streams are in-order, so semantics hold)."""
    for fn in nc.m.functions:
        for blk in fn.blocks:
            new_insts = []
            for inst in blk.instructions:
                si = getattr(inst, "sync_info", None)
                lim = (
                    max_waits_evsem
                    if isinstance(inst, mybir.InstEventSemaphore)
                    else max_waits
                )
                if si is not None and si.on_wait and len(si.on_wait) > lim:
                    waits = list(si.on_wait)
                    for w in waits[:-lim]:
                        new_insts.append(
                            mybir.InstNoOp(
                                name=nc.get_next_instruction_name(),
                                engine=inst.engine,
                                bass_nofuse=True,
                                sync_info=mybir.SyncInfo(on_wait=[w], on_update=[]),
                            )
                        )
                    si.on_wait = waits[-lim:]
                new_insts.append(inst)
            blk.instructions = new_insts


def _build_mha(nc: bass.Bass):
    qT = nc.dram_tensor("qT", [D, S], BF16, kind="ExternalInput")
    kT = nc.dram_tensor("kT", [D, S], BF16, kind="ExternalInput")
    vT = nc.dram_tensor("vT", [D, S], BF16, kind="ExternalInput")
    wq = nc.dram_tensor("wq", [D, P], BF16, kind="ExternalInput")
    wk = nc.dram_tensor("wk", [D, P], BF16, kind="ExternalInput")
    wv = nc.dram_tensor("wv", [D, P], BF16, kind="ExternalInput")
    wo = nc.dram_tensor("wo", [P, D], BF16, kind="ExternalInput")
    y = nc.dram_tensor("y", [S, D], F32, kind="ExternalOutput")

    qT3 = qT.rearrange("(o p) s -> p o s", p=P)
    kT3 = kT.rearrange("(o p) s -> p o s", p=P)
    vT3 = vT.rearrange("(o p) s -> p o s", p=P)
    wq3 = wq.rearrange("(o p) m -> p o m", p=P)
    wk3 = wk.rearrange("(o p) m -> p o m", p=P)
    wv3 = wv.rearrange("(o p) m -> p o m", p=P)

    with tile.TileContext(nc) as tc, contextlib.ExitStack() as ctx:
        static = ctx.enter_context(tc.tile_pool(name="static", bufs=1))
        xpool = ctx.enter_context(tc.tile_pool(name="x", bufs=8))
        vtmp = ctx.enter_context(tc.tile_pool(name="vtmp", bufs=2))
        ptp = ctx.enter_context(tc.tile_pool(name="pt", bufs=6))
        recp = ctx.enter_context(tc.tile_pool(name="rec", bufs=3))
        ystage = ctx.enter_context(tc.tile_pool(name="ystage", bufs=6))
        qk_ps = ctx.enter_context(tc.tile_pool(name="qkps", bufs=2, space="PSUM"))
        pv_ps = ctx.enter_context(tc.tile_pool(name="pvps", bufs=2, space="PSUM"))
        post_ps = ctx.enter_context(tc.tile_pool(name="postps", bufs=2, space="PSUM"))

        wq_sb = static.tile([P, NDC, P], BF16, tag="wq")
        wk_sb = static.tile([P, NDC, P], BF16, tag="wk")
        wv_sb = static.tile([P, NDC, P], BF16, tag="wv")
        wo_sb = static.tile([P, D], BF16, tag="wo")
        ident = static.tile([P, P], BF16, tag="ident")
        onesb = static.tile([DK + 1, DK], BF16, tag="onesb")
        khT = static.tile([P, S], BF16, tag="khT")
        qhT = static.tile([P, S], BF16, tag="qhT")
        vh = static.tile([P, NT, 2 * DK + 2], BF16, tag="vh")
        hc = static.tile([P, S], BF16, tag="hc")

        nc.gpsimd.memset(onesb[DK : DK + 1, :], 1.0)
        nc.sync.dma_start(wq_sb[:], wq3)
        nc.sync.dma_start(wk_sb[:], wk3)
        make_identity(nc, ident[:])
        # No explicit HAM warm-up: the strip-0 projections start as soon as
        # their DMA lands and serve as the warm-up themselves.

        def one_proj(jw, w_sb, src3, dst, half=None, xx_cache=None):
            """Projection of one 512-wide strip; half=0/1 emits only 4 of the
            8 contraction chunks (finer drip grains for strip 0's loop)."""

            def _th():
                if xx_cache is not None and xx_cache[0] is not None:
                    xx = xx_cache[0]
                else:
                    xx = xpool.tile([P, NDC, W], BF16, tag="xs")
                    nc.sync.dma_start(xx[:], src3[:, :, jw])
                    if xx_cache is not None:
                        xx_cache[0] = xx
                if xx_cache is not None and len(xx_cache) > 1 and xx_cache[1] is not None:
                    pp = xx_cache[1]
                else:
                    pp = post_ps.tile([P, W], F32, tag="post")
                    if xx_cache is not None and len(xx_cache) > 1:
                        xx_cache[1] = pp
                cs = range(NDC) if half is None else range(4 * half, 4 * half + 4)
                for c in cs:
                    nc.tensor.matmul(
                        pp[:], w_sb[:, c, :], xx[:, c, :],
                        start=(c == 0), stop=(c == NDC - 1),
                    )
                if half is None or half == 1:
                    nc.vector.tensor_copy(dst, pp[:])
            return _th

        def proj_halves(jw, w_sb, src3, dst):
            cache = [None, None]
            return [
                one_proj(jw, w_sb, src3, dst, half=0, xx_cache=cache),
                one_proj(jw, w_sb, src3, dst, half=1, xx_cache=cache),
            ]

        def v_thunks(j):
            """V projection + PE transpose for strip j, as 1+4 thunks."""
            jw = slice(j * W, (j + 1) * W)
            vts = vtmp.tile([P, W], BF16, tag="vts")

            def _tr(i):
                def _th():
                    t = j * TPW + i
                    ptr = post_ps.tile([P, P], BF16, tag="post")
                    nc.tensor.transpose(ptr[:], vts[:, i * P : (i + 1) * P], ident[:])
                    nc.vector.tensor_copy(vh[:, t, 0:DK], ptr[:, 0:DK])
                    nc.vector.tensor_copy(
                        vh[:, t, DK + 1 : 2 * DK + 1], ptr[:, DK : 2 * DK]
                    )
                return _th

            return [one_proj(jw, wv_sb, vT3, vts[:])] + [_tr(i) for i in range(TPW)]

        for t in range(NT):
            nc.gpsimd.memset(vh[:, t, DK : DK + 1], 1.0)
            nc.gpsimd.memset(vh[:, t, 2 * DK + 1 : 2 * DK + 2], 1.0)

        def q_thunks(j):
            jw = slice(j * W, (j + 1) * W)
            return proj_halves(jw, wq_sb, qT3, qhT[:, jw])

        # ---- strip 0 projections (before the loop) ----
        # q0/k0 use half-split DMAs so the first 4 contraction chunks start
        # multiplying while the second half of the strip is still in flight.
        def proj0_split(w_sb, src3, dst):
            xa = xpool.tile([P, 4, W], BF16, tag="xs0")
            xb = xpool.tile([P, 4, W], BF16, tag="xs0")
            nc.sync.dma_start(xa[:], src3[:, 0:4, 0:W])
            nc.sync.dma_start(xb[:], src3[:, 4:NDC, 0:W])
            pp = post_ps.tile([P, W], F32, tag="post")
            for c in range(NDC):
                xx = xa[:, c, :] if c < 4 else xb[:, c - 4, :]
                nc.tensor.matmul(
                    pp[:], w_sb[:, c, :], xx, start=(c == 0), stop=(c == NDC - 1)
                )
            nc.vector.tensor_copy(dst, pp[:])

        jw0 = slice(0, W)
        proj0_split(wq_sb, qT3, qhT[:, jw0])
        proj0_split(wk_sb, kT3, khT[:, jw0])
        nc.sync.dma_start(wv_sb[:], wv3)
        for th in v_thunks(0):
            th()

        def wo_th():
            nc.sync.dma_start(wo_sb[:], wo[:])

        # strip 0 drip: k/v for strips 1-7 (needed within strip 0's loop),
        # then wo and q for strip 1. q for strips 2-7 drip inside strips 1-6.
        drip0 = []
        for j in range(1, NS):
            jw = slice(j * W, (j + 1) * W)
            drip0 += proj_halves(jw, wk_sb, kT3, khT[:, jw])
            drip0 += v_thunks(j)
            if j == 2:
                drip0.append(wo_th)
        drip0 += q_thunks(1)

        def make_epilogue(s, lnd, final):
            """Deferred post-softmax work for strip s, drip-fed into the next
            strip's attention loop (fills engine slack under the exp pace)."""
            cw = slice(s * W, (s + 1) * W)
            thunks = []
            rec = recp.tile([DK + 1, 2 * W], BF16, tag="rec")
            bch = [None]

            def rec_th():
                # 1/d for both heads in one ScalarE pass: rec = exp(-ln d)
                with nc.allow_low_precision(
                    reason="bf16 softmax denominators feed a bf16 matmul broadcast"
                ):
                    nc.scalar.activation(
                        rec[DK : DK + 1, :], lnd[DK : DK + 1, :], EXP, scale=-1.0
                    )

            def sel_th():
                bc = post_ps.tile([P, W], F32, tag="post")
                bch[0] = bc
                nc.tensor.matmul(
                    bc[0:DK, :], onesb[DK : DK + 1, :], rec[DK : DK + 1, 0:W],
                    start=True, stop=True,
                )
                nc.tensor.matmul(
                    bc[DK:P, :], onesb[DK : DK + 1, :], rec[DK : DK + 1, W : 2 * W],
                    start=True, stop=True, tile_position=(DK, DK),
                )

            def norm_a():
                nc.vector.tensor_tensor(
                    hc[0:DK, cw], hc[0:DK, cw], bch[0][0:DK, :], op=MULT
                )

            def norm_b():
                nc.vector.tensor_tensor(
                    hc[DK:P, cw], hc[DK:P, cw], bch[0][DK:P, :], op=MULT
                )

            def norm_chunk(i):
                def _th():
                    cwi = slice(s * W + i * P, s * W + (i + 1) * P)
                    ci = slice(i * P, (i + 1) * P)
                    nc.vector.tensor_tensor(
                        hc[0:DK, cwi], hc[0:DK, cwi], bch[0][0:DK, ci], op=MULT
                    )
                    nc.vector.tensor_tensor(
                        hc[DK:P, cwi], hc[DK:P, cwi], bch[0][DK:P, ci], op=MULT
                    )
                return _th

            def proj_out(i, oh):
                def _th():
                    sq = s * TPW + i
                    py = post_ps.tile([P, W], F32, tag="post")
                    nc.tensor.matmul(
                        py[:],
                        hc[:, sq * P : (sq + 1) * P],
                        wo_sb[:, oh * W : (oh + 1) * W],
                        start=True, stop=True,
                    )
                    ys = ystage.tile([P, W], F32, tag="ys")
                    # final strip: exp stream is over, ScalarE is idle — let
                    # it share the PSUM evacuations so the tail is shorter
                    if final and (i + oh) % 2 == 0:
                        nc.scalar.copy(ys[:], py[:])
                    else:
                        nc.vector.tensor_copy(ys[:], py[:])
                    nc.sync.dma_start(
                        y[sq * P : (sq + 1) * P, oh * W : (oh + 1) * W], ys[:]
                    )
                return _th

            if final:
                # tail: per-sq-tile normalize so each output projection
                # starts as soon as its own columns are normalized
                thunks = [rec_th, sel_th]
                for i in range(TPW):
                    thunks += [norm_chunk(i), proj_out(i, 0), proj_out(i, 1)]
            else:
                thunks = [rec_th, sel_th, norm_a, norm_b]
                for i in range(TPW):
                    for oh in range(2):
                        thunks.append(proj_out(i, oh))
            return thunks

        # ---- main software-pipelined loop ----
        # QK pairs are emitted two tiles ahead of their exp so both PSUM
        # score buffers stay full: the ScalarE and VectorE exp streams then
        # run concurrently instead of serializing on the PE's in-order queue
        # (exp_t -> PV_t -> QK_{t+1} -> exp_{t+1} would otherwise be a
        # ~1.1us dependency ring regardless of engine assignment).
        def emit_qk(cw, t):
            qk = qk_ps.tile([P, 2 * W], F32, tag="qk")
            nc.tensor.matmul(
                qk[0:P, 0:W],
                khT[0:DK, t * P : (t + 1) * P], qhT[0:DK, cw],
                start=True, stop=True,
            )
            nc.tensor.matmul(
                qk[0:P, W : 2 * W],
                khT[DK:P, t * P : (t + 1) * P], qhT[DK:P, cw],
                start=True, stop=True,
            )
            return qk

        pending = []
        qk_ahead = []
        for s in range(NS):
            cw = slice(s * W, (s + 1) * W)
            pva = pv_ps.tile([P, W], F32, tag="pv")
            pvb = pv_ps.tile([P, W], F32, tag="pv")
            dve_tiles = _DVE_TILES if s > 0 else frozenset()
            if s == 0:
                qk_ahead.append(emit_qk(cw, 0))

            def emit_pv(t, pt):
                nc.tensor.matmul(
                    pva[0 : DK + 1, :],
                    vh[:, t, 0 : DK + 1], pt[:, 0:W],
                    start=(t == 0), stop=(t == NT - 1),
                )
                nc.tensor.matmul(
                    pvb[0 : DK + 1, :],
                    vh[:, t, DK + 1 : 2 * DK + 2], pt[:, W : 2 * W],
                    start=(t == 0), stop=(t == NT - 1),
                )

            # PV emission lags two tiles so it never sits in the PE's
            # in-order queue ahead of a QK pair while waiting on its exp:
            # accumulation order into pva/pvb is commutative.
            pv_queue = []
            for t in range(NT):
                qk = qk_ahead.pop(0)
                pt = ptp.tile([P, 2 * W], BF16, tag="pt")
                if t in dve_tiles:
                    with nc.allow_low_precision(
                        reason="schraudolph exp2 bit-trick on a minority of sk tiles"
                    ):
                        nc.vector.tensor_scalar(
                            pt[:].bitcast(I16), qk[:], SEXP_A, SEXP_B, MULT, ADD
                        )
                else:
                    nc.scalar.activation(pt[:], qk[:], EXP, scale=SCALE)
                # refill the score pipeline (crossing into the next strip)
                nt = t + 1
                if nt < NT:
                    qk_ahead.append(emit_qk(cw, nt))
                elif s + 1 < NS:
                    qk_ahead.append(
                        emit_qk(slice((s + 1) * W, (s + 2) * W), 0)
                    )
                pv_queue.append((t, pt))
                if len(pv_queue) > 2:
                    emit_pv(*pv_queue.pop(0))
                if s == 0:
                    for _ in range(2):
                        if drip0:
                            drip0.pop(0)()
                else:
                    if pending and t % 2 == 0:
                        pending.pop(0)()
                    if t == NT - 2:
                        while pending:
                            pending.pop(0)()
            while pv_queue:
                emit_pv(*pv_queue.pop(0))

            # strip boundary: evacuate PSUM fast (head A direct to hc, head B
            # via a bf16 stage + partition-shift DMA), denominators via Ln on
            # ScalarE straight from the PSUM rows — all off the exp path.
            tmpb = recp.tile([DK, W], BF16, tag="tmpb")
            nc.vector.tensor_copy(hc[0:DK, cw], pva[0:DK, :])
            nc.vector.tensor_copy(tmpb[:], pvb[0:DK, :])
            nc.sync.dma_start(hc[DK:P, cw], tmpb[:])
            lnd = recp.tile([DK + 1, 2 * W], F32, tag="lnd")
            nc.scalar.activation(lnd[DK : DK + 1, 0:W], pva[DK : DK + 1, :], LN)
            nc.scalar.activation(lnd[DK : DK + 1, W : 2 * W], pvb[DK : DK + 1, :], LN)

            assert not drip0 or s == 0
            newpend = make_epilogue(s, lnd, final=(s == NS - 1))
            if s + 1 < NS - 1:
                newpend = q_thunks(s + 2) + newpend
            assert not pending
            pending = newpend

        for th in pending:
            th()
    return nc


def _make_core_inputs(q, k, v, Wq, Wk, Wv, Wo, core, cache):
    bf = ml_dtypes.bfloat16
    if "qT" not in cache:
        cache["qT"] = np.ascontiguousarray(q.T).astype(bf)
        cache["kT"] = np.ascontiguousarray(k.T).astype(bf)
        cache["vT"] = np.ascontiguousarray(v.T).astype(bf)
    h0, h1 = 2 * core, 2 * core + 1
    return {
        "qT": cache["qT"],
        "kT": cache["kT"],
        "vT": cache["vT"],
        "wq": np.concatenate([Wq[h0], Wq[h1]], axis=1).astype(bf),
        "wk": np.concatenate([Wk[h0], Wk[h1]], axis=1).astype(bf),
        "wv": np.concatenate([Wv[h0], Wv[h1]], axis=1).astype(bf),
        "wo": np.ascontiguousarray(Wo[:, P * core : P * (core + 1)].T).astype(bf),
    }


_NC = None
last_results = None  # BassKernelResults of the most recent run (for profiling)


def _get_nc():
    global _NC
    if _NC is None:
        nc = bass.Bass("TRN2", target_bir_lowering=False, debug=False)
        _build_mha(nc)
        _split_excess_waits(nc)
        _NC = nc
    return _NC


def kernel(q, k, v, Wq, Wk, Wv, Wo, **run_kwargs):
    """Full-input MHA forward. Shards over 8 NeuronCores (2 heads each),
    runs the Bass kernel, and all-reduces the per-core partial outputs."""
    from concourse.bass_utils import run_bass_kernel_spmd

    global last_results
    q = np.asarray(q, np.float32)
    k = np.asarray(k, np.float32)
    v = np.asarray(v, np.float32)
    Wq = np.asarray(Wq, np.float32)
    Wk = np.asarray(Wk, np.float32)
    Wv = np.asarray(Wv, np.float32)
    Wo = np.asarray(Wo, np.float32)

    nc = _get_nc()
    cache = {}
    in_maps = [
        _make_core_inputs(q, k, v, Wq, Wk, Wv, Wo, c, cache) for c in range(NCORES)
    ]
    res = run_bass_kernel_spmd(
        nc, in_maps, core_ids=list(range(NCORES)), **run_kwargs
    )
    last_results = res
    y = res.results[0]["y"].astype(np.float32)
    for c in range(1, NCORES):
        y += res.results[c]["y"]
    return y
